# revision 50
# baseline (speedup 1.0000x reference)
"""Trainium2 Bass kernel for DSSConv2d via 1D Winograd F(2,3) along W.

Computation (per reference):
  convs = conv2d(x, w.reshape(rank*oc, ic, 3, 3), pad=1)   # [B, rank*oc, H, W]
  cw    = softmax(cw_row + cw_col, axis=0)                 # [rank, H, W]
  out   = einsum('bkcxy,kxy->bcxy', convs.reshape(B,rank,oc,H,W), cw)
  out  += b + b_col + b_row

Strategy:
  - Data parallel: batch 32 -> 4 images per core on 8 cores.
  - 3x3 conv = 3 row-taps of a 1D 3-tap conv along W, computed with Winograd
    F(2,3): 4 transformed components m=0..3, each a matmul
    tw[m,dy][ic,oc] @ t_m[ic, rows, j] accumulated over dy in PSUM.
    12 matmuls per rank-image instead of 18 direct-equivalent.
  - The Winograd input transform t0..t3 is computed on the HOST (free) and
    DMA'd in directly -- no DVE work and no DVE->matmul dependency at startup.
  - m2 weights are negated on the host so the output transform collapses to
    two interleaved fp16 DVE ops over [OC, 2, PIX2]:
      s_eo = add(cm[:,0:2,:], cm[:,1:3,:])   # (c0+c1, c1-c2)
      y_eo = sub(s_eo,        cm[:,2:4,:])   # (ye=se+c2, yo=so-c3)
    then combine: m_eo = y_eo * cw[r]; acc += m_eo  (bias folded into r=0).
  - PSUM tiles [OC, 4, 512] f32 (4 banks, one bank per m) so the Act engine
    drains a whole rank-block's 4 components in ONE strided copy.
  - Last image's last rank runs per-block (drain -> 4 small DVE ops -> DMA)
    so the tail after the final matmul is ~2.5us.
"""

import numpy as np
import ml_dtypes
from contextlib import ExitStack

import concourse.bass as bass
import concourse.mybir as mybir
import concourse.tile as tile
from concourse import bacc
from concourse.bass_utils import run_bass_kernel_spmd

RANK, OC, IC = 4, 128, 128
B, H, W = 32, 56, 56
NCORES = 8
B_LOC = B // NCORES          # 4 images per core
HP = H + 2                   # 58 padded rows
J = W // 2                   # 28 tile columns per row
PIX2 = H * J                 # 1568 pixels per phase

BF16 = mybir.dt.bfloat16
F16 = mybir.dt.float16
F32 = mybir.dt.float32

_CACHE = {}


def _build_nc():
    nc = bacc.Bacc()
    # host-transformed Winograd input planes: [img][ic][m][row][j]
    t_in = nc.dram_tensor("t", [B_LOC, IC, 4, HP, J], BF16,
                          kind="ExternalInput")
    # transformed weights [rank][ic][m][dy][oc], m=2 negated
    w_in = nc.dram_tensor("w", [RANK, IC, 4, 3, OC], BF16,
                          kind="ExternalInput")
    # softmax combine tables [rank][e/o][pix]
    cw_in = nc.dram_tensor("cw", [RANK, 2, PIX2], F16, kind="ExternalInput")
    bias_in = nc.dram_tensor("bias", [OC, 2, PIX2], F16,
                             kind="ExternalInput")
    out = nc.dram_tensor("out", [B_LOC, OC, 2, PIX2], F16,
                         kind="ExternalOutput")

    with tile.TileContext(nc) as tc, ExitStack() as ctx:
        consts = ctx.enter_context(tc.tile_pool(name="consts", bufs=1))
        tpool = ctx.enter_context(tc.tile_pool(name="tpool", bufs=2))
        cpool = ctx.enter_context(tc.tile_pool(name="cpool", bufs=2))
        # s/y/m scratch single-buffered: DVE consumes each in-order
        ypool = ctx.enter_context(tc.tile_pool(name="ypool", bufs=1))
        apool = ctx.enter_context(tc.tile_pool(name="apool", bufs=2))
        # each psum tile = 4 banks (one per Winograd component); 2 bufs = all 8
        pspool = ctx.enter_context(tc.tile_pool(name="ps", bufs=2,
                                                space="PSUM"))

        # equal 14-row blocks: per-block matmul time (~2.0us) always covers
        # the previous block's Act drain (~1.45us), so the 2-deep psum pool
        # never stalls the PE at block/rank boundaries
        BLKS = [(0, 14), (14, 14), (28, 14), (42, 14)]
        # t loaded in row chunks so the first matmuls start early
        TCH = [(0, 18), (18, 16), (34, 24)]

        # All input DMAs ride the sync ring; the Act ring carries only
        # out-DMAs so psum drains are never queued behind descriptor gen.
        # chunk boundaries aligned to the 14-row matmul blocks: block k of
        # the first rank only needs chunks 0..k, so the first matmuls start
        # as early as possible
        TCH0 = [(0, 16), (16, 14), (30, 14), (44, 14)]

        def load_img(img, chunks=TCH):
            tt = tpool.tile([IC, 4, HP, J], BF16, name=f"t{img}", tag="t")
            for r0, nr in chunks:
                for m in range(4):
                    nc.sync.dma_start(out=tt[:, m, r0:r0 + nr, :],
                                      in_=t_in[img][:, m, r0:r0 + nr, :])
            return tt

        w_sb = [consts.tile([IC, 4, 3, OC], BF16, name=f"w{r}")
                for r in range(RANK)]
        bias_sb = consts.tile([OC, 2, PIX2], F16, name="bias")
        cw_sb = [consts.tile([128, 2, PIX2], F16, name=f"cw{r}")
                 for r in range(RANK)]

        # Preamble DMA issue order tracks first-use order on the engines:
        # w0 + t0[0..15] gate the first matmul.
        t_cur = tpool.tile([IC, 4, HP, J], BF16, name="t0", tag="t")
        for m in range(4):
            nc.sync.dma_start(out=w_sb[0][:, m], in_=w_in[0][:, m])
            r0, nr = TCH0[0]
            nc.sync.dma_start(out=t_cur[:, m, r0:r0 + nr, :],
                              in_=t_in[0][:, m, r0:r0 + nr, :])
        for ci in (1, 2, 3):
            r0, nr = TCH0[ci]
            for m in range(4):
                nc.sync.dma_start(out=t_cur[:, m, r0:r0 + nr, :],
                                  in_=t_in[0][:, m, r0:r0 + nr, :])
        nc.sync.dma_start(out=w_sb[1], in_=w_in[1])
        nc.sync.dma_start(out=cw_sb[0], in_=cw_in[0].partition_broadcast(128))
        nc.sync.dma_start(out=bias_sb, in_=bias_in[:])
        nc.sync.dma_start(out=w_sb[2], in_=w_in[2])
        nc.sync.dma_start(out=cw_sb[1], in_=cw_in[1].partition_broadcast(128))
        nc.sync.dma_start(out=w_sb[3], in_=w_in[3])
        for r in (2, 3):
            nc.sync.dma_start(out=cw_sb[r],
                              in_=cw_in[r].partition_broadcast(128))

        # --- PE warmup: keep p-state high through the DMA preamble ---
        warm = consts.tile([128, 448], BF16, name="warm")
        nc.vector.memset(warm, 0.0)
        wps = pspool.tile([OC, 4, 512], F32, name="wps", tag="ps")
        for _ in range(4):
            nc.tensor.matmul(wps[:, 0, :448], lhsT=warm[:, :128], rhs=warm,
                             start=True, stop=True)
        wsink = consts.tile([128, 1], F32, name="wsink")
        nc.vector.tensor_copy(wsink, wps[:, 0, 0:1])

        def combine_chain(r, img, cm, acc_in, dst, q0, qn, dma):
            # output transform + eager rank combine on pixel range [q0,q0+qn)
            sl = (slice(None), slice(None), slice(q0, q0 + qn))
            s_ = ypool.tile([OC, 2, PIX2], F16, name="s", tag="s")
            y_ = ypool.tile([OC, 2, PIX2], F16, name="y", tag="y")
            m_ = ypool.tile([OC, 2, PIX2], F16, name="m", tag="m")
            nc.vector.tensor_add(s_[sl], cm[:, 0:2, q0:q0 + qn],
                                 cm[:, 1:3, q0:q0 + qn])
            nc.vector.tensor_sub(y_[sl], s_[sl], cm[:, 2:4, q0:q0 + qn])
            nc.vector.tensor_mul(m_[sl], y_[sl], cw_sb[r][sl])
            nc.vector.tensor_add(dst[sl], m_[sl],
                                 bias_sb[sl] if r == 0 else acc_in[sl])
            if dma:
                # scalar ring: PE consumes nothing issued there, so out-DMAs
                # can't stall the next image's matmuls
                nc.scalar.dma_start(out=out[img][:, :, q0:q0 + qn],
                                    in_=dst[sl])

        def process(img, r, t_sb, acc_in, acc_out, grp, dma,
                    split_last=False):
            # grp = #blocks per DVE combine chain; 0 = one full-plane chain.
            # Finer grp shortens the dependency tail after this rank's last
            # matmul at a small per-op overhead cost. split_last halves the
            # final block's drain+chain so the post-matmul tail is minimal.
            cm = cpool.tile([OC, 4, PIX2], F16, name="c", tag="c")
            q0 = 0
            for bi, (h0, nr) in enumerate(BLKS):
                last_b = bi + 1 == len(BLKS)
                nt = nr * J
                p0 = h0 * J
                ps = pspool.tile([OC, 4, 512], F32, name="ps", tag="ps")
                for m in range(4):
                    for dy in range(3):
                        nc.tensor.matmul(
                            ps[:, m, :nt],
                            lhsT=w_sb[r][:, m, dy, :],
                            rhs=t_sb[:, m, h0 + dy:h0 + dy + nr, :],
                            start=(dy == 0), stop=(dy == 2),
                        )
                if split_last and last_b:
                    h = nt // 2
                    nc.scalar.copy(cm[:, :, p0:p0 + h], ps[:, :, :h])
                    combine_chain(r, img, cm, acc_in, acc_out, p0, h,
                                  dma=dma)
                    nc.scalar.copy(cm[:, :, p0 + h:p0 + nt], ps[:, :, h:nt])
                    combine_chain(r, img, cm, acc_in, acc_out, p0 + h,
                                  nt - h, dma=dma)
                    continue
                nc.scalar.copy(cm[:, :, p0:p0 + nt], ps[:, :, :nt])
                if grp == 1:
                    combine_chain(r, img, cm, acc_in, acc_out, p0, nt,
                                  dma=dma)
                elif grp and (bi + 1) % grp == 0:
                    combine_chain(r, img, cm, acc_in, acc_out, q0,
                                  p0 + nt - q0, dma=dma)
                    q0 = p0 + nt
            if not grp:
                combine_chain(r, img, cm, acc_in, acc_out, 0, PIX2, dma=dma)

        for img in range(B_LOC):
            last_img = img + 1 == B_LOC
            if not last_img:
                t_nxt = load_img(img + 1)
            acc = None
            for r in range(RANK):
                last_r = r == RANK - 1
                if last_img:
                    grp = 1 if last_r else 2
                else:
                    grp = 2 if (img + 2 == B_LOC and last_r) else 0
                a_ = apool.tile([OC, 2, PIX2], F16, name=f"a{r}", tag="a")
                process(img, r, t_cur, acc, a_, grp=grp, dma=last_r,
                        split_last=last_img and last_r)
                acc = a_
            if not last_img:
                t_cur = t_nxt

    nc.finalize()
    return nc


def _prep_inputs(x, w, cw_row, cw_col, b_row, b_col, b):
    # host-side Winograd F(2,3) input transform on zero-padded input
    xp = np.zeros((B, IC, HP, HP), dtype=np.float32)
    xp[:, :, 1:H + 1, 1:W + 1] = x.astype(np.float32)
    xe0 = xp[:, :, :, 0:2 * J:2]       # d0 = cols 0,2,..,54
    xe1 = xp[:, :, :, 2:2 * J + 2:2]   # d2 = cols 2,4,..,56
    xo0 = xp[:, :, :, 1:2 * J + 1:2]   # d1 = cols 1,3,..,55
    xo1 = xp[:, :, :, 3:2 * J + 3:2]   # d3 = cols 3,5,..,57
    t = np.stack([xe0 - xe1, xo0 + xe1, xe1 - xo0, xo0 - xo1], axis=2)
    t = np.ascontiguousarray(t).astype(ml_dtypes.bfloat16)  # [B,IC,4,58,28]

    # Winograd-transformed weights tw[r, ic, m, dy, oc]; m=2 negated so the
    # device output transform is two interleaved add/sub ops
    G = np.array([[1, 0, 0], [.5, .5, .5], [.5, -.5, .5], [0, 0, 1]],
                 dtype=np.float64)
    tw = np.einsum("mx,rciyx->rimyc", G, w.astype(np.float64))
    tw[:, :, 2] = -tw[:, :, 2]
    tw = np.ascontiguousarray(tw).astype(ml_dtypes.bfloat16)

    # softmax over rank of per-pixel combine logits, split even/odd
    logits = (cw_row + cw_col).astype(np.float64)  # [rank, H, W]
    logits -= logits.max(axis=0, keepdims=True)
    e = np.exp(logits)
    cw = e / e.sum(axis=0, keepdims=True)
    cw2 = np.stack([cw[:, :, 0::2].reshape(RANK, PIX2),
                    cw[:, :, 1::2].reshape(RANK, PIX2)], axis=1)
    cw2 = cw2.astype(np.float16)  # [RANK, 2, PIX2]

    # combined bias plane split even/odd
    bias = (b.reshape(OC, 1, 1) + b_row.reshape(1, H, 1)
            + b_col.reshape(1, 1, W))
    bias2 = np.stack([bias[:, :, 0::2].reshape(OC, PIX2),
                      bias[:, :, 1::2].reshape(OC, PIX2)], axis=1)
    bias2 = bias2.astype(np.float16)  # [OC, 2, PIX2]

    return t, tw, cw2, bias2


def _run(inputs, trace=False):
    if "nc" not in _CACHE:
        _CACHE["nc"] = _build_nc()
    nc = _CACHE["nc"]
    t, tw, cw2, bias2 = _prep_inputs(**inputs)
    in_maps = [
        {"t": t[c * B_LOC:(c + 1) * B_LOC], "w": tw, "cw": cw2,
         "bias": bias2}
        for c in range(NCORES)
    ]
    res = run_bass_kernel_spmd(nc, in_maps, list(range(NCORES)), trace=trace)
    outs = [np.asarray(res.results[c]["out"]) for c in range(NCORES)]
    oall = np.concatenate(outs, axis=0)  # [B, OC, 2, PIX2] fp16
    full = np.empty((B, OC, H, W), dtype=np.float32)
    full[:, :, :, 0::2] = oall[:, :, 0].reshape(B, OC, H, J).astype(np.float32)
    full[:, :, :, 1::2] = oall[:, :, 1].reshape(B, OC, H, J).astype(np.float32)
    return full, res


def kernel(**inputs):
    full, _ = _run(inputs)
    return full


# revision 53
# speedup vs baseline: 1.0446x; 1.0446x over previous
"""Trainium2 Bass kernel for DSSConv2d via 1D Winograd F(2,3) along W.

Computation (per reference):
  convs = conv2d(x, w.reshape(rank*oc, ic, 3, 3), pad=1)   # [B, rank*oc, H, W]
  cw    = softmax(cw_row + cw_col, axis=0)                 # [rank, H, W]
  out   = einsum('bkcxy,kxy->bcxy', convs.reshape(B,rank,oc,H,W), cw)
  out  += b + b_col + b_row

Strategy:
  - Data parallel: batch 32 -> 4 images per core on 8 cores.
  - 3x3 conv = 3 row-taps of a 1D 3-tap conv along W, computed with Winograd
    F(2,3): 4 transformed components m=0..3, each a matmul
    tw[m,dy][ic,oc] @ t_m[ic, rows, j] accumulated over dy in PSUM.
    12 matmuls per rank-image instead of 18 direct-equivalent.
  - The Winograd input transform t0..t3 is computed on the HOST (free) and
    DMA'd in directly -- no DVE work and no DVE->matmul dependency at startup.
  - m2 weights are negated on the host so the output transform collapses to
    two interleaved fp16 DVE ops over [OC, 2, PIX2]:
      s_eo = add(cm[:,0:2,:], cm[:,1:3,:])   # (c0+c1, c1-c2)
      y_eo = sub(s_eo,        cm[:,2:4,:])   # (ye=se+c2, yo=so-c3)
    then combine: m_eo = y_eo * cw[r]; acc += m_eo  (bias folded into r=0).
  - PSUM tiles [OC, 4, 512] f32 (4 banks, one bank per m) so the Act engine
    drains a whole rank-block's 4 components in ONE strided copy.
  - Last image's last rank runs per-block (drain -> 4 small DVE ops -> DMA)
    so the tail after the final matmul is ~2.5us.
"""

import numpy as np
import ml_dtypes
from contextlib import ExitStack

import concourse.bass as bass
import concourse.mybir as mybir
import concourse.tile as tile
from concourse import bacc
from concourse.bass_utils import run_bass_kernel_spmd

RANK, OC, IC = 4, 128, 128
B, H, W = 32, 56, 56
NCORES = 8
B_LOC = B // NCORES          # 4 images per core
HP = H + 2                   # 58 padded rows
J = W // 2                   # 28 tile columns per row
PIX2 = H * J                 # 1568 pixels per phase

BF16 = mybir.dt.bfloat16
F16 = mybir.dt.float16
F32 = mybir.dt.float32

_CACHE = {}


def _build_nc():
    nc = bacc.Bacc()
    # host-transformed Winograd input planes: [img][ic][m][row][j]
    t_in = nc.dram_tensor("t", [B_LOC, IC, 4, HP, J], BF16,
                          kind="ExternalInput")
    # transformed weights [rank][ic][m][dy][oc], m=2 negated
    w_in = nc.dram_tensor("w", [RANK, IC, 4, 3, OC], BF16,
                          kind="ExternalInput")
    # softmax combine tables [rank][e/o][pix]
    cw_in = nc.dram_tensor("cw", [RANK, 2, PIX2], F16, kind="ExternalInput")
    bias_in = nc.dram_tensor("bias", [OC, 2, PIX2], F16,
                             kind="ExternalInput")
    out = nc.dram_tensor("out", [B_LOC, OC, 2, PIX2], F16,
                         kind="ExternalOutput")

    with tile.TileContext(nc) as tc, ExitStack() as ctx:
        consts = ctx.enter_context(tc.tile_pool(name="consts", bufs=1))
        # 3 buffers on t and a: a full extra image/rank of slack between the
        # prefetch DMA writes and the previous tiles' in-flight readers
        tpool = ctx.enter_context(tc.tile_pool(name="tpool", bufs=3))
        cpool = ctx.enter_context(tc.tile_pool(name="cpool", bufs=2))
        # s/y/m scratch single-buffered: DVE consumes each in-order
        ypool = ctx.enter_context(tc.tile_pool(name="ypool", bufs=1))
        apool = ctx.enter_context(tc.tile_pool(name="apool", bufs=3))
        # each psum tile = 4 banks (one per Winograd component); 2 bufs = all 8
        pspool = ctx.enter_context(tc.tile_pool(name="ps", bufs=2,
                                                space="PSUM"))

        # equal 14-row blocks: per-block matmul time (~2.0us) always covers
        # the previous block's Act drain (~1.45us), so the 2-deep psum pool
        # never stalls the PE at block/rank boundaries
        BLKS = [(0, 14), (14, 14), (28, 14), (42, 14)]
        # t loaded in row chunks so the first matmuls start early
        TCH = [(0, 18), (18, 16), (34, 24)]

        # All input DMAs ride the sync ring; the Act ring carries only
        # out-DMAs so psum drains are never queued behind descriptor gen.
        # chunk boundaries aligned to the 14-row matmul blocks: block k of
        # the first rank only needs chunks 0..k, so the first matmuls start
        # as early as possible
        TCH0 = [(0, 16), (16, 14), (30, 14), (44, 14)]

        def load_img(img, chunks=TCH):
            tt = tpool.tile([IC, 4, HP, J], BF16, name=f"t{img}", tag="t")
            for r0, nr in chunks:
                for m in range(4):
                    nc.sync.dma_start(out=tt[:, m, r0:r0 + nr, :],
                                      in_=t_in[img][:, m, r0:r0 + nr, :])
            return tt

        w_sb = [consts.tile([IC, 4, 3, OC], BF16, name=f"w{r}")
                for r in range(RANK)]
        bias_sb = consts.tile([OC, 2, PIX2], F16, name="bias")
        cw_sb = [consts.tile([128, 2, PIX2], F16, name=f"cw{r}")
                 for r in range(RANK)]

        # Preamble DMA issue order tracks first-use order on the engines:
        # w0 + t0[0..17] gate the first matmul.
        t_cur = tpool.tile([IC, 4, HP, J], BF16, name="t0", tag="t")
        for m in range(4):
            nc.sync.dma_start(out=w_sb[0][:, m], in_=w_in[0][:, m])
            r0, nr = TCH[0]
            nc.sync.dma_start(out=t_cur[:, m, r0:r0 + nr, :],
                              in_=t_in[0][:, m, r0:r0 + nr, :])
        for ci in (1, 2):
            r0, nr = TCH[ci]
            for m in range(4):
                nc.sync.dma_start(out=t_cur[:, m, r0:r0 + nr, :],
                                  in_=t_in[0][:, m, r0:r0 + nr, :])
        nc.sync.dma_start(out=w_sb[1], in_=w_in[1])
        nc.sync.dma_start(out=cw_sb[0], in_=cw_in[0].partition_broadcast(128))
        nc.sync.dma_start(out=bias_sb, in_=bias_in[:])
        nc.sync.dma_start(out=w_sb[2], in_=w_in[2])
        nc.sync.dma_start(out=cw_sb[1], in_=cw_in[1].partition_broadcast(128))
        nc.sync.dma_start(out=w_sb[3], in_=w_in[3])
        for r in (2, 3):
            nc.sync.dma_start(out=cw_sb[r],
                              in_=cw_in[r].partition_broadcast(128))

        # --- PE warmup: keep p-state high through the DMA preamble ---
        warm = consts.tile([128, 448], BF16, name="warm")
        nc.vector.memset(warm, 0.0)
        wps = pspool.tile([OC, 4, 512], F32, name="wps", tag="ps")
        for _ in range(6):
            nc.tensor.matmul(wps[:, 0, :448], lhsT=warm[:, :128], rhs=warm,
                             start=True, stop=True)
        wsink = consts.tile([128, 1], F32, name="wsink")
        nc.vector.tensor_copy(wsink, wps[:, 0, 0:1])

        def combine_chain(r, img, cm, acc_in, dst, q0, qn, dma):
            # output transform + eager rank combine on pixel range [q0,q0+qn)
            sl = (slice(None), slice(None), slice(q0, q0 + qn))
            s_ = ypool.tile([OC, 2, PIX2], F16, name="s", tag="s")
            y_ = ypool.tile([OC, 2, PIX2], F16, name="y", tag="y")
            m_ = ypool.tile([OC, 2, PIX2], F16, name="m", tag="m")
            nc.vector.tensor_add(s_[sl], cm[:, 0:2, q0:q0 + qn],
                                 cm[:, 1:3, q0:q0 + qn])
            nc.vector.tensor_sub(y_[sl], s_[sl], cm[:, 2:4, q0:q0 + qn])
            nc.vector.tensor_mul(m_[sl], y_[sl], cw_sb[r][sl])
            nc.vector.tensor_add(dst[sl], m_[sl],
                                 bias_sb[sl] if r == 0 else acc_in[sl])
            if dma:
                # scalar ring: PE consumes nothing issued there, so out-DMAs
                # can't stall the next image's matmuls
                nc.scalar.dma_start(out=out[img][:, :, q0:q0 + qn],
                                    in_=dst[sl])

        def process(img, r, t_sb, acc_in, acc_out, grp, dma,
                    split_last=False):
            # grp = #blocks per DVE combine chain; 0 = one full-plane chain.
            # Finer grp shortens the dependency tail after this rank's last
            # matmul at a small per-op overhead cost. split_last halves the
            # final block's drain+chain so the post-matmul tail is minimal.
            cm = cpool.tile([OC, 4, PIX2], F16, name="c", tag="c")
            q0 = 0
            for bi, (h0, nr) in enumerate(BLKS):
                last_b = bi + 1 == len(BLKS)
                nt = nr * J
                p0 = h0 * J
                ps = pspool.tile([OC, 4, 512], F32, name="ps", tag="ps")
                for m in range(4):
                    for dy in range(3):
                        nc.tensor.matmul(
                            ps[:, m, :nt],
                            lhsT=w_sb[r][:, m, dy, :],
                            rhs=t_sb[:, m, h0 + dy:h0 + dy + nr, :],
                            start=(dy == 0), stop=(dy == 2),
                        )
                if split_last and last_b:
                    h = nt // 2
                    nc.scalar.copy(cm[:, :, p0:p0 + h], ps[:, :, :h])
                    combine_chain(r, img, cm, acc_in, acc_out, p0, h,
                                  dma=dma)
                    nc.scalar.copy(cm[:, :, p0 + h:p0 + nt], ps[:, :, h:nt])
                    combine_chain(r, img, cm, acc_in, acc_out, p0 + h,
                                  nt - h, dma=dma)
                    continue
                nc.scalar.copy(cm[:, :, p0:p0 + nt], ps[:, :, :nt])
                if grp == 1:
                    combine_chain(r, img, cm, acc_in, acc_out, p0, nt,
                                  dma=dma)
                elif grp and (bi + 1) % grp == 0:
                    combine_chain(r, img, cm, acc_in, acc_out, q0,
                                  p0 + nt - q0, dma=dma)
                    q0 = p0 + nt
            if not grp:
                combine_chain(r, img, cm, acc_in, acc_out, 0, PIX2, dma=dma)

        for img in range(B_LOC):
            last_img = img + 1 == B_LOC
            if not last_img:
                t_nxt = load_img(img + 1)
            acc = None
            for r in range(RANK):
                last_r = r == RANK - 1
                if last_img:
                    grp = 1 if last_r else 2
                else:
                    grp = 2 if (img + 2 == B_LOC and last_r) else 0
                a_ = apool.tile([OC, 2, PIX2], F16, name=f"a{r}", tag="a")
                process(img, r, t_cur, acc, a_, grp=grp, dma=last_r,
                        split_last=last_img and last_r)
                acc = a_
            if not last_img:
                t_cur = t_nxt

    nc.finalize()
    return nc


def _prep_inputs(x, w, cw_row, cw_col, b_row, b_col, b):
    # host-side Winograd F(2,3) input transform on zero-padded input
    xp = np.zeros((B, IC, HP, HP), dtype=np.float32)
    xp[:, :, 1:H + 1, 1:W + 1] = x.astype(np.float32)
    xe0 = xp[:, :, :, 0:2 * J:2]       # d0 = cols 0,2,..,54
    xe1 = xp[:, :, :, 2:2 * J + 2:2]   # d2 = cols 2,4,..,56
    xo0 = xp[:, :, :, 1:2 * J + 1:2]   # d1 = cols 1,3,..,55
    xo1 = xp[:, :, :, 3:2 * J + 3:2]   # d3 = cols 3,5,..,57
    t = np.stack([xe0 - xe1, xo0 + xe1, xe1 - xo0, xo0 - xo1], axis=2)
    t = np.ascontiguousarray(t).astype(ml_dtypes.bfloat16)  # [B,IC,4,58,28]

    # Winograd-transformed weights tw[r, ic, m, dy, oc]; m=2 negated so the
    # device output transform is two interleaved add/sub ops
    G = np.array([[1, 0, 0], [.5, .5, .5], [.5, -.5, .5], [0, 0, 1]],
                 dtype=np.float64)
    tw = np.einsum("mx,rciyx->rimyc", G, w.astype(np.float64))
    tw[:, :, 2] = -tw[:, :, 2]
    tw = np.ascontiguousarray(tw).astype(ml_dtypes.bfloat16)

    # softmax over rank of per-pixel combine logits, split even/odd
    logits = (cw_row + cw_col).astype(np.float64)  # [rank, H, W]
    logits -= logits.max(axis=0, keepdims=True)
    e = np.exp(logits)
    cw = e / e.sum(axis=0, keepdims=True)
    cw2 = np.stack([cw[:, :, 0::2].reshape(RANK, PIX2),
                    cw[:, :, 1::2].reshape(RANK, PIX2)], axis=1)
    cw2 = cw2.astype(np.float16)  # [RANK, 2, PIX2]

    # combined bias plane split even/odd
    bias = (b.reshape(OC, 1, 1) + b_row.reshape(1, H, 1)
            + b_col.reshape(1, 1, W))
    bias2 = np.stack([bias[:, :, 0::2].reshape(OC, PIX2),
                      bias[:, :, 1::2].reshape(OC, PIX2)], axis=1)
    bias2 = bias2.astype(np.float16)  # [OC, 2, PIX2]

    return t, tw, cw2, bias2


def _run(inputs, trace=False):
    if "nc" not in _CACHE:
        _CACHE["nc"] = _build_nc()
    nc = _CACHE["nc"]
    t, tw, cw2, bias2 = _prep_inputs(**inputs)
    in_maps = [
        {"t": t[c * B_LOC:(c + 1) * B_LOC], "w": tw, "cw": cw2,
         "bias": bias2}
        for c in range(NCORES)
    ]
    res = run_bass_kernel_spmd(nc, in_maps, list(range(NCORES)), trace=trace)
    outs = [np.asarray(res.results[c]["out"]) for c in range(NCORES)]
    oall = np.concatenate(outs, axis=0)  # [B, OC, 2, PIX2] fp16
    full = np.empty((B, OC, H, W), dtype=np.float32)
    full[:, :, :, 0::2] = oall[:, :, 0].reshape(B, OC, H, J).astype(np.float32)
    full[:, :, :, 1::2] = oall[:, :, 1].reshape(B, OC, H, J).astype(np.float32)
    return full, res


def kernel(**inputs):
    full, _ = _run(inputs)
    return full


# revision 56
# speedup vs baseline: 1.0528x; 1.0078x over previous
"""Trainium2 Bass kernel for DSSConv2d via 1D Winograd F(2,3) along W.

Computation (per reference):
  convs = conv2d(x, w.reshape(rank*oc, ic, 3, 3), pad=1)   # [B, rank*oc, H, W]
  cw    = softmax(cw_row + cw_col, axis=0)                 # [rank, H, W]
  out   = einsum('bkcxy,kxy->bcxy', convs.reshape(B,rank,oc,H,W), cw)
  out  += b + b_col + b_row

Strategy:
  - Data parallel: batch 32 -> 4 images per core on 8 cores.
  - 3x3 conv = 3 row-taps of a 1D 3-tap conv along W, computed with Winograd
    F(2,3): 4 transformed components m=0..3, each a matmul
    tw[m,dy][ic,oc] @ t_m[ic, rows, j] accumulated over dy in PSUM.
    12 matmuls per rank-image instead of 18 direct-equivalent.
  - The Winograd input transform t0..t3 is computed on the HOST (free) and
    DMA'd in directly -- no DVE work and no DVE->matmul dependency at startup.
  - m2 weights are negated on the host so the output transform collapses to
    two interleaved fp16 DVE ops over [OC, 2, PIX2]:
      s_eo = add(cm[:,0:2,:], cm[:,1:3,:])   # (c0+c1, c1-c2)
      y_eo = sub(s_eo,        cm[:,2:4,:])   # (ye=se+c2, yo=so-c3)
    then combine: m_eo = y_eo * cw[r]; acc += m_eo  (bias folded into r=0).
  - PSUM tiles [OC, 4, 512] f32 (4 banks, one bank per m) so the Act engine
    drains a whole rank-block's 4 components in ONE strided copy.
  - Last image's last rank runs per-block (drain -> 4 small DVE ops -> DMA)
    so the tail after the final matmul is ~2.5us.
"""

import numpy as np
import ml_dtypes
from contextlib import ExitStack

import concourse.bass as bass
import concourse.mybir as mybir
import concourse.tile as tile
from concourse import bacc
from concourse.bass_utils import run_bass_kernel_spmd

RANK, OC, IC = 4, 128, 128
B, H, W = 32, 56, 56
NCORES = 8
B_LOC = B // NCORES          # 4 images per core
HP = H + 2                   # 58 padded rows
J = W // 2                   # 28 tile columns per row
PIX2 = H * J                 # 1568 pixels per phase

BF16 = mybir.dt.bfloat16
F16 = mybir.dt.float16
F32 = mybir.dt.float32

_CACHE = {}


def _build_nc():
    nc = bacc.Bacc()
    # host-transformed Winograd input planes: [img][ic][m][row][j]
    t_in = nc.dram_tensor("t", [B_LOC, IC, 4, HP, J], BF16,
                          kind="ExternalInput")
    # transformed weights [rank][ic][m][dy][oc], m=2 negated
    w_in = nc.dram_tensor("w", [RANK, IC, 4, 3, OC], BF16,
                          kind="ExternalInput")
    # combine tables / output in half-major layout [half][e/o][784]
    cw_in = nc.dram_tensor("cw", [RANK, 2, 2, 784], F16,
                           kind="ExternalInput")
    bias_in = nc.dram_tensor("bias", [OC, 2, 2, 784], F16,
                             kind="ExternalInput")
    out = nc.dram_tensor("out", [B_LOC, OC, 2, 2, 784], F16,
                         kind="ExternalOutput")

    with tile.TileContext(nc) as tc, ExitStack() as ctx:
        consts = ctx.enter_context(tc.tile_pool(name="consts", bufs=1))
        # 3 buffers on t and a: a full extra image/rank of slack between the
        # prefetch DMA writes and the previous tiles' in-flight readers
        tpool = ctx.enter_context(tc.tile_pool(name="tpool", bufs=3))
        cpool = ctx.enter_context(tc.tile_pool(name="cpool", bufs=2))
        # s/y/m scratch single-buffered: DVE consumes each in-order
        ypool = ctx.enter_context(tc.tile_pool(name="ypool", bufs=1))
        apool = ctx.enter_context(tc.tile_pool(name="apool", bufs=3))
        # each psum tile = 4 banks (one per Winograd component); 2 bufs = all 8
        pspool = ctx.enter_context(tc.tile_pool(name="ps", bufs=2,
                                                space="PSUM"))

        # equal 14-row blocks: per-block matmul time (~2.0us) always covers
        # the previous block's Act drain (~1.45us), so the 2-deep psum pool
        # never stalls the PE at block/rank boundaries
        BLKS = [(0, 14), (14, 14), (28, 14), (42, 14)]
        # t loaded in row chunks so the first matmuls start early
        TCH = [(0, 18), (18, 16), (34, 24)]

        # All input DMAs ride the sync ring; the Act ring carries only
        # out-DMAs so psum drains are never queued behind descriptor gen.
        # chunk boundaries aligned to the 14-row matmul blocks: block k of
        # the first rank only needs chunks 0..k, so the first matmuls start
        # as early as possible
        TCH0 = [(0, 16), (16, 14), (30, 14), (44, 14)]

        def load_img(img, chunks=TCH):
            tt = tpool.tile([IC, 4, HP, J], BF16, name=f"t{img}", tag="t")
            for r0, nr in chunks:
                for m in range(4):
                    nc.sync.dma_start(out=tt[:, m, r0:r0 + nr, :],
                                      in_=t_in[img][:, m, r0:r0 + nr, :])
            return tt

        w_sb = [consts.tile([IC, 4, 3, OC], BF16, name=f"w{r}")
                for r in range(RANK)]
        bias_sb = consts.tile([OC, 2, 2, 784], F16, name="bias")
        cw_sb = [consts.tile([128, 2, 2, 784], F16, name=f"cw{r}")
                 for r in range(RANK)]

        # Preamble DMA issue order tracks first-use order on the engines:
        # w0 + t0[0..17] gate the first matmul.
        t_cur = tpool.tile([IC, 4, HP, J], BF16, name="t0", tag="t")
        for m in range(4):
            nc.sync.dma_start(out=w_sb[0][:, m], in_=w_in[0][:, m])
            r0, nr = TCH[0]
            nc.sync.dma_start(out=t_cur[:, m, r0:r0 + nr, :],
                              in_=t_in[0][:, m, r0:r0 + nr, :])
        for ci in (1, 2):
            r0, nr = TCH[ci]
            for m in range(4):
                nc.sync.dma_start(out=t_cur[:, m, r0:r0 + nr, :],
                                  in_=t_in[0][:, m, r0:r0 + nr, :])
        nc.sync.dma_start(out=w_sb[1], in_=w_in[1])
        nc.sync.dma_start(out=cw_sb[0], in_=cw_in[0].partition_broadcast(128))
        nc.sync.dma_start(out=bias_sb, in_=bias_in[:])
        nc.sync.dma_start(out=w_sb[2], in_=w_in[2])
        nc.sync.dma_start(out=cw_sb[1], in_=cw_in[1].partition_broadcast(128))
        nc.sync.dma_start(out=w_sb[3], in_=w_in[3])
        for r in (2, 3):
            nc.sync.dma_start(out=cw_sb[r],
                              in_=cw_in[r].partition_broadcast(128))

        # --- PE warmup: keep p-state high through the DMA preamble ---
        warm = consts.tile([128, 448], BF16, name="warm")
        nc.vector.memset(warm, 0.0)
        wps = pspool.tile([OC, 4, 512], F32, name="wps", tag="ps")
        for _ in range(6):
            nc.tensor.matmul(wps[:, 0, :448], lhsT=warm[:, :128], rhs=warm,
                             start=True, stop=True)
        wsink = consts.tile([128, 1], F32, name="wsink")
        nc.vector.tensor_copy(wsink, wps[:, 0, 0:1])

        def combine_chain(r, img, cm, acc_in, dst, q0, qn, dma):
            # Output transform + eager rank combine on pixel range [q0,q0+qn)
            # of the half-major planes. Ranges aligned to 784-px halves give
            # fully contiguous DVE access patterns (no strided-AP penalty).
            h0_, h1_ = q0 // 784, (q0 + qn + 783) // 784
            o0, on = q0 - h0_ * 784, None
            if h1_ - h0_ == 1:
                sl = (slice(None), h0_, slice(None),
                      slice(o0, o0 + qn))
                cmsl = lambda a, b: cm[:, h0_, a:b, o0:o0 + qn]
                cwsl = cw_sb[r][:, h0_, :, o0:o0 + qn]
                bsl = bias_sb[:, h0_, :, o0:o0 + qn]
                outsl = out[img][:, h0_, :, o0:o0 + qn]
            else:
                sl = (slice(None), slice(h0_, h1_), slice(None),
                      slice(None))
                cmsl = lambda a, b: cm[:, h0_:h1_, a:b, :]
                cwsl = cw_sb[r][:, h0_:h1_]
                bsl = bias_sb[:, h0_:h1_]
                outsl = out[img][:, h0_:h1_]
            s_ = ypool.tile([OC, 2, 2, 784], F16, name="s", tag="s")
            y_ = ypool.tile([OC, 2, 2, 784], F16, name="y", tag="y")
            m_ = ypool.tile([OC, 2, 2, 784], F16, name="m", tag="m")
            nc.vector.tensor_add(s_[sl], cmsl(0, 2), cmsl(1, 3))
            nc.vector.tensor_sub(y_[sl], s_[sl], cmsl(2, 4))
            nc.vector.tensor_mul(m_[sl], y_[sl], cwsl)
            nc.vector.tensor_add(dst[sl], m_[sl],
                                 bsl if r == 0 else acc_in[sl])
            if dma:
                # scalar ring: PE consumes nothing issued there, so out-DMAs
                # can't stall the next image's matmuls
                nc.scalar.dma_start(out=outsl, in_=dst[sl])

        def process(img, r, t_sb, acc_in, acc_out, grp, dma,
                    split_last=False):
            # grp = #blocks per DVE combine chain; 0 = one full-plane chain.
            # Finer grp shortens the dependency tail after this rank's last
            # matmul at a small per-op overhead cost. split_last halves the
            # final block's drain+chain so the post-matmul tail is minimal.
            cm = cpool.tile([OC, 2, 4, 784], F16, name="c", tag="c")
            q0 = 0
            for bi, (h0, nr) in enumerate(BLKS):
                last_b = bi + 1 == len(BLKS)
                nt = nr * J
                p0 = h0 * J
                ps = pspool.tile([OC, 4, 512], F32, name="ps", tag="ps")
                for m in range(4):
                    for dy in range(3):
                        nc.tensor.matmul(
                            ps[:, m, :nt],
                            lhsT=w_sb[r][:, m, dy, :],
                            rhs=t_sb[:, m, h0 + dy:h0 + dy + nr, :],
                            start=(dy == 0), stop=(dy == 2),
                        )
                hf, off = p0 // 784, p0 % 784
                if split_last and last_b:
                    h = nt // 2
                    nc.scalar.copy(cm[:, hf, :, off:off + h], ps[:, :, :h])
                    combine_chain(r, img, cm, acc_in, acc_out, p0, h,
                                  dma=dma)
                    nc.scalar.copy(cm[:, hf, :, off + h:off + nt],
                                   ps[:, :, h:nt])
                    combine_chain(r, img, cm, acc_in, acc_out, p0 + h,
                                  nt - h, dma=dma)
                    continue
                nc.scalar.copy(cm[:, hf, :, off:off + nt], ps[:, :, :nt])
                if grp == 1:
                    combine_chain(r, img, cm, acc_in, acc_out, p0, nt,
                                  dma=dma)
                elif grp and (bi + 1) % grp == 0:
                    combine_chain(r, img, cm, acc_in, acc_out, q0,
                                  p0 + nt - q0, dma=dma)
                    q0 = p0 + nt
            if not grp:
                combine_chain(r, img, cm, acc_in, acc_out, 0, PIX2, dma=dma)

        for img in range(B_LOC):
            last_img = img + 1 == B_LOC
            if not last_img:
                t_nxt = load_img(img + 1)
            acc = None
            for r in range(RANK):
                last_r = r == RANK - 1
                if last_img:
                    grp = 1 if last_r else 2
                else:
                    grp = 2 if (img + 2 == B_LOC and last_r) else 0
                a_ = apool.tile([OC, 2, 2, 784], F16, name=f"a{r}",
                                tag="a")
                process(img, r, t_cur, acc, a_, grp=grp, dma=last_r,
                        split_last=last_img and last_r)
                acc = a_
            if not last_img:
                t_cur = t_nxt

    nc.finalize()
    return nc


def _prep_inputs(x, w, cw_row, cw_col, b_row, b_col, b):
    # host-side Winograd F(2,3) input transform on zero-padded input
    xp = np.zeros((B, IC, HP, HP), dtype=np.float32)
    xp[:, :, 1:H + 1, 1:W + 1] = x.astype(np.float32)
    xe0 = xp[:, :, :, 0:2 * J:2]       # d0 = cols 0,2,..,54
    xe1 = xp[:, :, :, 2:2 * J + 2:2]   # d2 = cols 2,4,..,56
    xo0 = xp[:, :, :, 1:2 * J + 1:2]   # d1 = cols 1,3,..,55
    xo1 = xp[:, :, :, 3:2 * J + 3:2]   # d3 = cols 3,5,..,57
    t = np.stack([xe0 - xe1, xo0 + xe1, xe1 - xo0, xo0 - xo1], axis=2)
    t = np.ascontiguousarray(t).astype(ml_dtypes.bfloat16)  # [B,IC,4,58,28]

    # Winograd-transformed weights tw[r, ic, m, dy, oc]; m=2 negated so the
    # device output transform is two interleaved add/sub ops
    G = np.array([[1, 0, 0], [.5, .5, .5], [.5, -.5, .5], [0, 0, 1]],
                 dtype=np.float64)
    tw = np.einsum("mx,rciyx->rimyc", G, w.astype(np.float64))
    tw[:, :, 2] = -tw[:, :, 2]
    tw = np.ascontiguousarray(tw).astype(ml_dtypes.bfloat16)

    # softmax over rank of per-pixel combine logits, split even/odd
    logits = (cw_row + cw_col).astype(np.float64)  # [rank, H, W]
    logits -= logits.max(axis=0, keepdims=True)
    e = np.exp(logits)
    cw = e / e.sum(axis=0, keepdims=True)
    cw2 = np.stack([cw[:, :, 0::2].reshape(RANK, PIX2),
                    cw[:, :, 1::2].reshape(RANK, PIX2)], axis=1)
    # half-major: [RANK, 2(half), 2(e/o), 784]
    cw2 = np.ascontiguousarray(
        cw2.reshape(RANK, 2, 2, 784).transpose(0, 2, 1, 3))
    cw2 = cw2.astype(np.float16)

    # combined bias plane split even/odd
    bias = (b.reshape(OC, 1, 1) + b_row.reshape(1, H, 1)
            + b_col.reshape(1, 1, W))
    bias2 = np.stack([bias[:, :, 0::2].reshape(OC, PIX2),
                      bias[:, :, 1::2].reshape(OC, PIX2)], axis=1)
    bias2 = np.ascontiguousarray(
        bias2.reshape(OC, 2, 2, 784).transpose(0, 2, 1, 3))
    bias2 = bias2.astype(np.float16)  # [OC, 2(half), 2(e/o), 784]

    return t, tw, cw2, bias2


def _run(inputs, trace=False):
    if "nc" not in _CACHE:
        _CACHE["nc"] = _build_nc()
    nc = _CACHE["nc"]
    t, tw, cw2, bias2 = _prep_inputs(**inputs)
    in_maps = [
        {"t": t[c * B_LOC:(c + 1) * B_LOC], "w": tw, "cw": cw2,
         "bias": bias2}
        for c in range(NCORES)
    ]
    res = run_bass_kernel_spmd(nc, in_maps, list(range(NCORES)), trace=trace)
    outs = [np.asarray(res.results[c]["out"]) for c in range(NCORES)]
    oall = np.concatenate(outs, axis=0)  # [B, OC, 2half, 2eo, 784] fp16
    full = np.empty((B, OC, H, W), dtype=np.float32)
    full[:, :, :, 0::2] = oall[:, :, :, 0].reshape(B, OC, H, J)        .astype(np.float32)
    full[:, :, :, 1::2] = oall[:, :, :, 1].reshape(B, OC, H, J)        .astype(np.float32)
    return full, res


def kernel(**inputs):
    full, _ = _run(inputs)
    return full
